# revision 5
# baseline (speedup 1.0000x reference)
"""Distributed causal multi-head attention for TRN2 (8 NeuronCores).

Sharding: tensor-parallel over heads — core i computes heads {2i, 2i+1}
(128 of the 1024 hidden dims) for the whole (batch, seq) = (4, 2048).
All attention runs in transposed layouts (Q^T/K^T as [dh, t], scores as
[k, q]) so no on-chip transposes are needed except a cheap V^T -> V pass.
A single 8-rank AllToAll re-shards from head-parallel to token-parallel
before the output projection: core i ends up with the full 1024-dim
attention output for tokens [i*1024, (i+1)*1024) of the flattened
(8192,) token axis and computes that slice of out = attn @ W_o^T.

Matmuls run as float32r (single-pass fp32 on the PE, ~4x plain fp32).
"""

import sys

sys.path.insert(0, "/opt/trn_rl_repo")

import numpy as np

import concourse.bass as bass
import concourse.tile as tile
from concourse import bacc, mybir
from concourse.bass_utils import run_bass_kernel_spmd
from concourse.masks import make_identity

F32 = mybir.dt.float32
F32R = mybir.dt.float32r

B, S, D = 4, 2048, 1024
N_HEAD, D_HEAD = 16, 64
T = B * S               # 8192 flattened tokens
N_CORES = 8
HPC = N_HEAD // N_CORES  # heads per core = 2
E = HPC * D_HEAD         # 128 local attn dims per core
TW = 512                 # token window for QKV phase
QM = 512                 # query macro-tile for attention
KT = 128                 # key tile
TOK = T // N_CORES       # 1024 tokens owned per core after A2A
SCALE = 1.0 / 8.0        # 1/sqrt(64)
NEG = -1e9

TRACE = False
LAST_EXEC_NS = None
_CACHED_NC = None


def _build():
    nc = bacc.Bacc("TRN2", target_bir_lowering=False, debug=False,
                   num_devices=N_CORES)
    xT = nc.dram_tensor("xT", [D, T], F32R, kind="ExternalInput").ap()
    wqkvT = nc.dram_tensor("wqkvT", [D, 3 * E], F32R, kind="ExternalInput").ap()
    woT = nc.dram_tensor("woT", [D, D], F32R, kind="ExternalInput").ap()
    out = nc.dram_tensor("out", [TOK, D], F32, kind="ExternalOutput").ap()
    cc_in = nc.dram_tensor("cc_in", [N_CORES, E, TOK], F32R).ap()
    cc_out = nc.dram_tensor("cc_out", [N_CORES, E, TOK], F32R).ap()
    rg = [list(range(N_CORES))]

    n_dt = D // 128          # 8 d-tiles (contraction for projections)
    n_tw = T // TW           # 16 token windows
    n_kt = T // KT           # 64 key tiles total

    with tile.TileContext(nc) as tc:
        with (
            tc.tile_pool(name="persist", bufs=1) as pp,
            tc.tile_pool(name="weights", bufs=1) as wp,
        ):
            # ---- persistent SBUF ----
            qt = pp.tile([E, T], F32R, tag="qt")          # Q^T [128, 8192]
            kt_sb = pp.tile([E, T], F32R, tag="kt")       # K^T [128, 8192]
            vb = pp.tile([128, n_kt, 2 * (D_HEAD + 1)], F32R, tag="vb")
            wk_sb = wp.tile([128, n_dt, 3 * E], F32R, tag="wk")
            for dt in range(n_dt):
                nc.sync.dma_start(out=wk_sb[:, dt, :],
                                  in_=wqkvT[dt * 128:(dt + 1) * 128, :])

            # masks for the 4 diagonal k-tile offsets + identity + ones
            masks = []
            for j in range(4):
                m = wp.tile([KT, QM], F32, tag=f"mask{j}")
                nc.gpsimd.memset(m, 0.0)
                nc.gpsimd.affine_select(
                    out=m, in_=m, compare_op=mybir.AluOpType.is_ge,
                    fill=NEG, base=-(j * KT),
                    pattern=[[1, QM]], channel_multiplier=-1,
                )
                masks.append(m)
            ident_f = wp.tile([128, 128], F32, tag="idf")
            make_identity(nc, ident_f)
            ident = wp.tile([128, 128], F32R, tag="idr")
            nc.vector.tensor_copy(ident, ident_f)
            ones_f = wp.tile([128, n_kt], F32, tag="ones")
            nc.vector.memset(ones_f, 1.0)
            ones_row = wp.tile([1, D_HEAD], F32R, tag="ones_r")
            nc.vector.tensor_copy(ones_row, ones_f[0:1, 0:D_HEAD])

            # ---- phase A: QKV projections (all in transposed layout) ----
            with (
                tc.tile_pool(name="psA", bufs=3, space="PSUM") as psA,
                tc.tile_pool(name="spA", bufs=2) as sp,
                tc.tile_pool(name="scrA", bufs=2) as scr,
            ):
                for tw in range(n_tw):
                    xw = sp.tile([128, n_dt, TW], F32R, tag="xw")
                    for dt in range(n_dt):
                        nc.sync.dma_start(
                            out=xw[:, dt, :],
                            in_=xT[dt * 128:(dt + 1) * 128,
                                   tw * TW:(tw + 1) * TW])
                    for which, dst in ((0, qt), (1, kt_sb)):
                        ps = psA.tile([128, TW], F32, tag="ps_qkv")
                        for dt in range(n_dt):
                            nc.tensor.matmul(
                                ps[0:E, :],
                                wk_sb[:, dt, which * E:(which + 1) * E],
                                xw[:, dt, :],
                                start=(dt == 0), stop=(dt == n_dt - 1))
                        nc.vector.tensor_copy(
                            dst[:, tw * TW:(tw + 1) * TW], ps[0:E, :])
                    # V^T then transpose into natural layout with ones column
                    ps = psA.tile([128, TW], F32, tag="ps_qkv")
                    for dt in range(n_dt):
                        nc.tensor.matmul(
                            ps[0:E, :], wk_sb[:, dt, 2 * E:3 * E],
                            xw[:, dt, :],
                            start=(dt == 0), stop=(dt == n_dt - 1))
                    vt_sb = scr.tile([E, TW], F32R, tag="vt")
                    nc.vector.tensor_copy(vt_sb, ps[0:E, :])
                    for sub in range(TW // 128):   # 4 k-tiles per window
                        k_idx = tw * (TW // 128) + sub
                        pst = psA.tile([128, 128], F32R, tag="ps_vt")
                        nc.tensor.transpose(
                            pst[:, 0:E], vt_sb[:, sub * 128:(sub + 1) * 128],
                            ident)
                        nc.vector.tensor_copy(
                            vb[:, k_idx, :].rearrange(
                                "p (h c) -> p h c", c=D_HEAD + 1)[:, :, 0:D_HEAD],
                            pst[:, 0:E].rearrange(
                                "p (h c) -> p h c", c=D_HEAD))
                # ones columns (col 64 and 129 of every k-tile)
                for h in range(HPC):
                    nc.vector.tensor_copy(
                        vb[:, :, h * (D_HEAD + 1) + D_HEAD:
                               h * (D_HEAD + 1) + D_HEAD + 1],
                        ones_f[:, :, None])

            # ---- phase B: causal attention, transposed flash-style ----
            with (
                tc.tile_pool(name="psS", bufs=3, space="PSUM") as psS,
                tc.tile_pool(name="psO", bufs=2, space="PSUM") as psO,
                tc.tile_pool(name="psBc", bufs=2, space="PSUM") as psBc,
                tc.tile_pool(name="scrBp", bufs=3) as scr,
                tc.tile_pool(name="scrB2", bufs=2) as scr2,
            ):
                for b in range(B):
                    for h in range(HPC):
                        hp = h * D_HEAD
                        for qm in range(S // QM):
                            q0 = b * S + qm * QM
                            n_k = 4 * qm + 4
                            ps_o = psO.tile([128, QM], F32, tag="ps_o")
                            for k in range(n_k):
                                k0 = b * S + k * KT
                                ps_s = psS.tile([KT, QM], F32, tag="ps_s")
                                nc.tensor.matmul(
                                    ps_s[:, :],
                                    kt_sb[hp:hp + D_HEAD, k0:k0 + KT],
                                    qt[hp:hp + D_HEAD, q0:q0 + QM],
                                    start=True, stop=True)
                                pt = scr.tile([KT, QM], F32R, tag="pt")
                                if k >= 4 * qm:  # diagonal: apply causal mask
                                    nc.vector.scalar_tensor_tensor(
                                        out=pt, in0=ps_s, scalar=SCALE,
                                        in1=masks[k - 4 * qm],
                                        op0=mybir.AluOpType.mult,
                                        op1=mybir.AluOpType.add)
                                    nc.scalar.activation(
                                        out=pt, in_=pt,
                                        func=mybir.ActivationFunctionType.Exp)
                                else:
                                    nc.scalar.activation(
                                        out=pt, in_=ps_s,
                                        func=mybir.ActivationFunctionType.Exp,
                                        scale=SCALE)
                                nc.tensor.matmul(
                                    ps_o[0:D_HEAD + 1, :],
                                    vb[:, b * (S // KT) + k,
                                       h * (D_HEAD + 1):(h + 1) * (D_HEAD + 1)],
                                    pt,
                                    start=(k == 0), stop=(k == n_k - 1))
                            # normalize: row D_HEAD holds the denominator
                            recip = scr2.tile([1, QM], F32R, tag="recip")
                            with nc.allow_low_precision(reason="f32r recip"):
                                nc.vector.reciprocal(
                                    recip, ps_o[D_HEAD:D_HEAD + 1, :])
                            ps_b = psBc.tile([D_HEAD, QM], F32, tag="ps_b")
                            nc.tensor.matmul(
                                ps_b[:, :], ones_row, recip,
                                start=True, stop=True)
                            num_sb = scr2.tile([D_HEAD, QM], F32R, tag="num")
                            nc.vector.tensor_copy(num_sb, ps_o[0:D_HEAD, :])
                            attn_sb = scr2.tile([D_HEAD, QM], F32R, tag="attn")
                            nc.vector.tensor_mul(attn_sb, num_sb, ps_b)
                            shard = b * 2 + qm // 2
                            qc = (qm % 2) * QM
                            nc.sync.dma_start(
                                out=cc_in[shard, hp:hp + D_HEAD, qc:qc + QM],
                                in_=attn_sb)

            # ---- phase C: A2A reshard + output projection ----
            nc.gpsimd.collective_compute(
                "AllToAll", mybir.AluOpType.bypass,
                ins=[cc_in.opt()], outs=[cc_out.opt()], replica_groups=rg)

            with (
                tc.tile_pool(name="psC", bufs=3, space="PSUM") as psC,
                tc.tile_pool(name="wpC", bufs=1) as wpc,
                tc.tile_pool(name="spC", bufs=2) as sp,
                tc.tile_pool(name="scrC", bufs=2) as scr,
            ):
                wo_sb = wpc.tile([128, n_dt, D], F32R, tag="wo")
                for dt in range(n_dt):
                    nc.sync.dma_start(out=wo_sb[:, dt, :],
                                      in_=woT[dt * 128:(dt + 1) * 128, :])
                for tt in range(TOK // 128):
                    a_sb = sp.tile([128, n_dt, 128], F32R, tag="a")
                    for dt in range(n_dt):
                        nc.sync.dma_start(
                            out=a_sb[:, dt, :],
                            in_=cc_out[dt, :, tt * 128:(tt + 1) * 128])
                    for et in range(D // 512):
                        ps = psC.tile([128, 512], F32, tag="ps_c")
                        for dt in range(n_dt):
                            nc.tensor.matmul(
                                ps[:, :], a_sb[:, dt, :],
                                wo_sb[:, dt, et * 512:(et + 1) * 512],
                                start=(dt == 0), stop=(dt == n_dt - 1))
                        o_sb = scr.tile([128, 512], F32, tag="o")
                        nc.vector.tensor_copy(o_sb, ps)
                        nc.sync.dma_start(
                            out=out[tt * 128:(tt + 1) * 128,
                                    et * 512:(et + 1) * 512],
                            in_=o_sb)
    nc.compile()
    return nc


def kernel(x, W_qkv, W_o):
    global _CACHED_NC, LAST_EXEC_NS
    if _CACHED_NC is None:
        _CACHED_NC = _build()
    nc = _CACHED_NC

    x = np.ascontiguousarray(x, dtype=np.float32)
    xT = np.ascontiguousarray(x.reshape(T, D).T)          # (1024, 8192)
    woT = np.ascontiguousarray(W_o.astype(np.float32).T)  # (1024, 1024)
    in_maps = []
    for i in range(N_CORES):
        rows = []
        for blk in range(3):                               # Q, K, V blocks
            rows.append(W_qkv[blk * D + i * E: blk * D + (i + 1) * E, :])
        wqkvT = np.ascontiguousarray(
            np.concatenate(rows, axis=0).astype(np.float32).T)  # (1024, 384)
        in_maps.append({"xT": xT, "wqkvT": wqkvT, "woT": woT})

    res = run_bass_kernel_spmd(nc, in_maps, core_ids=list(range(N_CORES)),
                               trace=TRACE)
    LAST_EXEC_NS = res.exec_time_ns
    full = np.empty((T, D), dtype=np.float32)
    for i in range(N_CORES):
        full[i * TOK:(i + 1) * TOK, :] = res.results[i]["out"]
    return full.reshape(B, S, D)


# revision 6
# speedup vs baseline: 1.0159x; 1.0159x over previous
"""Distributed causal multi-head attention for TRN2 (8 NeuronCores).

Sharding: tensor-parallel over heads — core i computes heads {2i, 2i+1}
(128 of the 1024 hidden dims) for the whole (batch, seq) = (4, 2048).
All attention runs in transposed layouts (Q^T/K^T as [dh, t], scores as
[k, q]) so no on-chip transposes are needed except a cheap V^T -> V pass.
A single 8-rank AllToAll re-shards from head-parallel to token-parallel
before the output projection: core i ends up with the full 1024-dim
attention output for tokens [i*1024, (i+1)*1024) of the flattened
(8192,) token axis and computes that slice of out = attn @ W_o^T.

Matmuls run as float32r (single-pass fp32 on the PE, ~4x plain fp32).
"""

import sys

sys.path.insert(0, "/opt/trn_rl_repo")

import numpy as np

import concourse.bass as bass
import concourse.tile as tile
from concourse import bacc, mybir
from concourse.bass_utils import run_bass_kernel_spmd
from concourse.masks import make_identity

F32 = mybir.dt.float32
F32R = mybir.dt.float32r

B, S, D = 4, 2048, 1024
N_HEAD, D_HEAD = 16, 64
T = B * S               # 8192 flattened tokens
N_CORES = 8
HPC = N_HEAD // N_CORES  # heads per core = 2
E = HPC * D_HEAD         # 128 local attn dims per core
TW = 512                 # token window for QKV phase
QM = 512                 # query macro-tile for attention
KT = 128                 # key tile
TOK = T // N_CORES       # 1024 tokens owned per core after A2A
SCALE = 1.0 / 8.0        # 1/sqrt(64)
NEG = -1e9

TRACE = False
LAST_EXEC_NS = None
_CACHED_NC = None


def _build():
    nc = bacc.Bacc("TRN2", target_bir_lowering=False, debug=False,
                   num_devices=N_CORES)
    xT = nc.dram_tensor("xT", [D, T], F32R, kind="ExternalInput").ap()
    wqkvT = nc.dram_tensor("wqkvT", [D, 3 * E], F32R, kind="ExternalInput").ap()
    woT = nc.dram_tensor("woT", [D, D], F32R, kind="ExternalInput").ap()
    out = nc.dram_tensor("out", [TOK, D], F32, kind="ExternalOutput").ap()
    cc_in = nc.dram_tensor("cc_in", [N_CORES, E, TOK], F32R).ap()
    cc_out = nc.dram_tensor("cc_out", [N_CORES, E, TOK], F32R).ap()
    rg = [list(range(N_CORES))]

    n_dt = D // 128          # 8 d-tiles (contraction for projections)
    n_tw = T // TW           # 16 token windows
    n_kt = T // KT           # 64 key tiles total

    with tile.TileContext(nc) as tc:
        with (
            tc.tile_pool(name="persist", bufs=1) as pp,
            tc.tile_pool(name="weights", bufs=1) as wp,
        ):
            # ---- persistent SBUF ----
            qt = pp.tile([E, T], F32R, tag="qt")          # Q^T [128, 8192]
            kt_sb = pp.tile([E, T], F32R, tag="kt")       # K^T [128, 8192]
            vb = pp.tile([128, n_kt, 2 * (D_HEAD + 1)], F32R, tag="vb")
            wk_sb = wp.tile([128, n_dt, 3 * E], F32R, tag="wk")
            for dt in range(n_dt):
                nc.sync.dma_start(out=wk_sb[:, dt, :],
                                  in_=wqkvT[dt * 128:(dt + 1) * 128, :])

            # masks for the 4 diagonal k-tile offsets + identity + ones
            masks = []
            for j in range(4):
                m = wp.tile([KT, QM], F32, tag=f"mask{j}")
                nc.gpsimd.memset(m, 0.0)
                nc.gpsimd.affine_select(
                    out=m, in_=m, compare_op=mybir.AluOpType.is_ge,
                    fill=NEG, base=-(j * KT),
                    pattern=[[1, QM]], channel_multiplier=-1,
                )
                masks.append(m)
            ident_f = wp.tile([128, 128], F32, tag="idf")
            make_identity(nc, ident_f)
            ident = wp.tile([128, 128], F32R, tag="idr")
            nc.vector.tensor_copy(ident, ident_f)
            ones_f = wp.tile([128, n_kt], F32, tag="ones")
            nc.vector.memset(ones_f, 1.0)
            ones_row = wp.tile([1, D_HEAD], F32R, tag="ones_r")
            nc.vector.tensor_copy(ones_row, ones_f[0:1, 0:D_HEAD])

            # ---- phase A: QKV projections (all in transposed layout) ----
            with (
                nc.named_scope("qkv"),
                tc.tile_pool(name="psA", bufs=3, space="PSUM") as psA,
                tc.tile_pool(name="spA", bufs=2) as sp,
                tc.tile_pool(name="scrA", bufs=2) as scr,
            ):
                for tw in range(n_tw):
                    xw = sp.tile([128, n_dt, TW], F32R, tag="xw")
                    for dt in range(n_dt):
                        nc.sync.dma_start(
                            out=xw[:, dt, :],
                            in_=xT[dt * 128:(dt + 1) * 128,
                                   tw * TW:(tw + 1) * TW])
                    for which, dst in ((0, qt), (1, kt_sb)):
                        ps = psA.tile([128, TW], F32, tag="ps_qkv")
                        for dt in range(n_dt):
                            nc.tensor.matmul(
                                ps[0:E, :],
                                wk_sb[:, dt, which * E:(which + 1) * E],
                                xw[:, dt, :],
                                start=(dt == 0), stop=(dt == n_dt - 1))
                        nc.vector.tensor_copy(
                            dst[:, tw * TW:(tw + 1) * TW], ps[0:E, :])
                    # V^T then transpose into natural layout with ones column
                    ps = psA.tile([128, TW], F32, tag="ps_qkv")
                    for dt in range(n_dt):
                        nc.tensor.matmul(
                            ps[0:E, :], wk_sb[:, dt, 2 * E:3 * E],
                            xw[:, dt, :],
                            start=(dt == 0), stop=(dt == n_dt - 1))
                    vt_sb = scr.tile([E, TW], F32R, tag="vt")
                    nc.vector.tensor_copy(vt_sb, ps[0:E, :])
                    for sub in range(TW // 128):   # 4 k-tiles per window
                        k_idx = tw * (TW // 128) + sub
                        pst = psA.tile([128, 128], F32R, tag="ps_vt")
                        nc.tensor.transpose(
                            pst[:, 0:E], vt_sb[:, sub * 128:(sub + 1) * 128],
                            ident)
                        nc.vector.tensor_copy(
                            vb[:, k_idx, :].rearrange(
                                "p (h c) -> p h c", c=D_HEAD + 1)[:, :, 0:D_HEAD],
                            pst[:, 0:E].rearrange(
                                "p (h c) -> p h c", c=D_HEAD))
                # ones columns (col 64 and 129 of every k-tile)
                for h in range(HPC):
                    nc.vector.tensor_copy(
                        vb[:, :, h * (D_HEAD + 1) + D_HEAD:
                               h * (D_HEAD + 1) + D_HEAD + 1],
                        ones_f[:, :, None])

            # ---- phase B: causal attention, transposed flash-style ----
            with (
                nc.named_scope("attn"),
                tc.tile_pool(name="psS", bufs=3, space="PSUM") as psS,
                tc.tile_pool(name="psO", bufs=2, space="PSUM") as psO,
                tc.tile_pool(name="psBc", bufs=2, space="PSUM") as psBc,
                tc.tile_pool(name="scrBp", bufs=3) as scr,
                tc.tile_pool(name="scrB2", bufs=2) as scr2,
            ):
                for b in range(B):
                    for h in range(HPC):
                        hp = h * D_HEAD
                        for qm in range(S // QM):
                            q0 = b * S + qm * QM
                            n_k = 4 * qm + 4
                            ps_o = psO.tile([128, QM], F32, tag="ps_o")
                            for k in range(n_k):
                                k0 = b * S + k * KT
                                ps_s = psS.tile([KT, QM], F32, tag="ps_s")
                                nc.tensor.matmul(
                                    ps_s[:, :],
                                    kt_sb[hp:hp + D_HEAD, k0:k0 + KT],
                                    qt[hp:hp + D_HEAD, q0:q0 + QM],
                                    start=True, stop=True)
                                pt = scr.tile([KT, QM], F32R, tag="pt")
                                if k >= 4 * qm:  # diagonal: apply causal mask
                                    nc.vector.scalar_tensor_tensor(
                                        out=pt, in0=ps_s, scalar=SCALE,
                                        in1=masks[k - 4 * qm],
                                        op0=mybir.AluOpType.mult,
                                        op1=mybir.AluOpType.add)
                                    nc.scalar.activation(
                                        out=pt, in_=pt,
                                        func=mybir.ActivationFunctionType.Exp)
                                else:
                                    nc.scalar.activation(
                                        out=pt, in_=ps_s,
                                        func=mybir.ActivationFunctionType.Exp,
                                        scale=SCALE)
                                nc.tensor.matmul(
                                    ps_o[0:D_HEAD + 1, :],
                                    vb[:, b * (S // KT) + k,
                                       h * (D_HEAD + 1):(h + 1) * (D_HEAD + 1)],
                                    pt,
                                    start=(k == 0), stop=(k == n_k - 1))
                            # normalize: row D_HEAD holds the denominator
                            recip = scr2.tile([1, QM], F32R, tag="recip")
                            with nc.allow_low_precision(reason="f32r recip"):
                                nc.vector.reciprocal(
                                    recip, ps_o[D_HEAD:D_HEAD + 1, :])
                            ps_b = psBc.tile([D_HEAD, QM], F32, tag="ps_b")
                            nc.tensor.matmul(
                                ps_b[:, :], ones_row, recip,
                                start=True, stop=True)
                            num_sb = scr2.tile([D_HEAD, QM], F32R, tag="num")
                            nc.vector.tensor_copy(num_sb, ps_o[0:D_HEAD, :])
                            attn_sb = scr2.tile([D_HEAD, QM], F32R, tag="attn")
                            nc.vector.tensor_mul(attn_sb, num_sb, ps_b)
                            shard = b * 2 + qm // 2
                            qc = (qm % 2) * QM
                            nc.sync.dma_start(
                                out=cc_in[shard, hp:hp + D_HEAD, qc:qc + QM],
                                in_=attn_sb)

            # ---- phase C: A2A reshard + output projection ----
            with nc.named_scope("a2a"):
                nc.gpsimd.collective_compute(
                    "AllToAll", mybir.AluOpType.bypass,
                    ins=[cc_in.opt()], outs=[cc_out.opt()], replica_groups=rg)

            with (
                nc.named_scope("oproj"),
                tc.tile_pool(name="psC", bufs=3, space="PSUM") as psC,
                tc.tile_pool(name="wpC", bufs=1) as wpc,
                tc.tile_pool(name="spC", bufs=2) as sp,
                tc.tile_pool(name="scrC", bufs=2) as scr,
            ):
                wo_sb = wpc.tile([128, n_dt, D], F32R, tag="wo")
                for dt in range(n_dt):
                    nc.sync.dma_start(out=wo_sb[:, dt, :],
                                      in_=woT[dt * 128:(dt + 1) * 128, :])
                for tt in range(TOK // 128):
                    a_sb = sp.tile([128, n_dt, 128], F32R, tag="a")
                    for dt in range(n_dt):
                        nc.sync.dma_start(
                            out=a_sb[:, dt, :],
                            in_=cc_out[dt, :, tt * 128:(tt + 1) * 128])
                    for et in range(D // 512):
                        ps = psC.tile([128, 512], F32, tag="ps_c")
                        for dt in range(n_dt):
                            nc.tensor.matmul(
                                ps[:, :], a_sb[:, dt, :],
                                wo_sb[:, dt, et * 512:(et + 1) * 512],
                                start=(dt == 0), stop=(dt == n_dt - 1))
                        o_sb = scr.tile([128, 512], F32, tag="o")
                        nc.vector.tensor_copy(o_sb, ps)
                        nc.sync.dma_start(
                            out=out[tt * 128:(tt + 1) * 128,
                                    et * 512:(et + 1) * 512],
                            in_=o_sb)
    nc.compile()
    return nc


def kernel(x, W_qkv, W_o):
    global _CACHED_NC, LAST_EXEC_NS
    if _CACHED_NC is None:
        _CACHED_NC = _build()
    nc = _CACHED_NC

    x = np.ascontiguousarray(x, dtype=np.float32)
    xT = np.ascontiguousarray(x.reshape(T, D).T)          # (1024, 8192)
    woT = np.ascontiguousarray(W_o.astype(np.float32).T)  # (1024, 1024)
    in_maps = []
    for i in range(N_CORES):
        rows = []
        for blk in range(3):                               # Q, K, V blocks
            rows.append(W_qkv[blk * D + i * E: blk * D + (i + 1) * E, :])
        wqkvT = np.ascontiguousarray(
            np.concatenate(rows, axis=0).astype(np.float32).T)  # (1024, 384)
        in_maps.append({"xT": xT, "wqkvT": wqkvT, "woT": woT})

    res = run_bass_kernel_spmd(nc, in_maps, core_ids=list(range(N_CORES)),
                               trace=TRACE)
    LAST_EXEC_NS = res.exec_time_ns
    kernel.LAST_RES = res
    full = np.empty((T, D), dtype=np.float32)
    for i in range(N_CORES):
        full[i * TOK:(i + 1) * TOK, :] = res.results[i]["out"]
    return full.reshape(B, S, D)


# revision 10
# speedup vs baseline: 1.1321x; 1.1144x over previous
"""Distributed causal multi-head attention for TRN2 (8 NeuronCores).

Sharding: tensor-parallel over heads — core i computes heads {2i, 2i+1}
(128 of the 1024 hidden dims) for the whole (batch, seq) = (4, 2048).
All attention runs in transposed layouts (Q^T/K^T as [dh, t], scores as
[k, q]) so no on-chip transposes are needed except a cheap V^T -> V pass.
A single 8-rank AllToAll re-shards from head-parallel to token-parallel
before the output projection: core i ends up with the full 1024-dim
attention output for tokens [i*1024, (i+1)*1024) of the flattened
(8192,) token axis and computes that slice of out = attn @ W_o^T.

Matmuls run as float32r (single-pass fp32 on the PE, ~4x plain fp32).
"""

import sys

sys.path.insert(0, "/opt/trn_rl_repo")

import numpy as np

import concourse.bass as bass
import concourse.tile as tile
from concourse import bacc, mybir
from concourse.bass_utils import run_bass_kernel_spmd
from concourse.masks import make_identity

F32 = mybir.dt.float32
F32R = mybir.dt.float32r
BF16 = mybir.dt.bfloat16

B, S, D = 4, 2048, 1024
N_HEAD, D_HEAD = 16, 64
T = B * S               # 8192 flattened tokens
N_CORES = 8
HPC = N_HEAD // N_CORES  # heads per core = 2
E = HPC * D_HEAD         # 128 local attn dims per core
TW = 512                 # token window for QKV phase
QM = 512                 # query macro-tile for attention
KT = 128                 # key tile
TOK = T // N_CORES       # 1024 tokens owned per core after A2A
SCALE = 1.0 / 8.0        # 1/sqrt(64)
NEG = -1e9

TRACE = False
LAST_EXEC_NS = None
_CACHED_NC = None


def _build():
    nc = bacc.Bacc("TRN2", target_bir_lowering=False, debug=False,
                   num_devices=N_CORES)
    xT = nc.dram_tensor("xT", [D, T], F32R, kind="ExternalInput").ap()
    wqkvT = nc.dram_tensor("wqkvT", [D, 3 * E], F32R, kind="ExternalInput").ap()
    woT = nc.dram_tensor("woT", [D, D], F32R, kind="ExternalInput").ap()
    out = nc.dram_tensor("out", [TOK, D], F32, kind="ExternalOutput").ap()
    cc_in = nc.dram_tensor("cc_in", [N_CORES, E, TOK], F32R).ap()
    cc_out = nc.dram_tensor("cc_out", [N_CORES, E, TOK], F32R).ap()
    rg = [list(range(N_CORES))]

    n_dt = D // 128          # 8 d-tiles (contraction for projections)
    n_tw = T // TW           # 16 token windows
    n_kt = T // KT           # 64 key tiles total

    with tile.TileContext(nc) as tc:
        with (
            tc.tile_pool(name="persist", bufs=1) as pp,
            tc.tile_pool(name="weights", bufs=1) as wp,
        ):
            # ---- persistent SBUF ----
            qt = pp.tile([E, T], BF16, tag="qt")          # Q^T [128, 8192]
            kt_sb = pp.tile([E, T], BF16, tag="kt")       # K^T [128, 8192]
            vb = pp.tile([128, n_kt, 2 * (D_HEAD + 1)], BF16, tag="vb")
            wk_sb = wp.tile([128, n_dt, 3 * E], F32R, tag="wk")
            for dt in range(n_dt):
                nc.sync.dma_start(out=wk_sb[:, dt, :],
                                  in_=wqkvT[dt * 128:(dt + 1) * 128, :])

            # masks for the 4 diagonal k-tile offsets + identity + ones
            masks = []
            for j in range(4):
                m = wp.tile([KT, QM], F32, tag=f"mask{j}")
                nc.gpsimd.memset(m, 0.0)
                nc.gpsimd.affine_select(
                    out=m, in_=m, compare_op=mybir.AluOpType.is_ge,
                    fill=NEG, base=-(j * KT),
                    pattern=[[1, QM]], channel_multiplier=-1,
                )
                masks.append(m)
            ident_f = wp.tile([128, 128], F32, tag="idf")
            make_identity(nc, ident_f)
            ident = wp.tile([128, 128], BF16, tag="idr")
            nc.vector.tensor_copy(ident, ident_f)
            ones_f = wp.tile([128, n_kt], F32, tag="ones")
            nc.vector.memset(ones_f, 1.0)
            ones_row = wp.tile([1, D_HEAD], F32, tag="ones_r")
            nc.vector.tensor_copy(ones_row, ones_f[0:1, 0:D_HEAD])
            wo_sb = wp.tile([128, n_dt, D], F32R, tag="wo")
            for dt in range(n_dt):
                nc.sync.dma_start(out=wo_sb[:, dt, :],
                                  in_=woT[dt * 128:(dt + 1) * 128, :])

            # ---- phase A: QKV projections (all in transposed layout) ----
            with (
                nc.named_scope("qkv"),
                tc.tile_pool(name="psA", bufs=3, space="PSUM") as psA,
                tc.tile_pool(name="spA", bufs=2) as sp,
                tc.tile_pool(name="scrA", bufs=2) as scr,
            ):
                # ones columns (col 64 and 129 of every k-tile) — written
                # first so attention tiles don't wait on the whole QKV phase
                for h in range(HPC):
                    nc.vector.tensor_copy(
                        vb[:, :, h * (D_HEAD + 1) + D_HEAD:
                               h * (D_HEAD + 1) + D_HEAD + 1],
                        ones_f[:, :, None])
                for tw in range(n_tw):
                    xw = sp.tile([128, n_dt, TW], F32R, tag="xw")
                    for dt in range(n_dt):
                        nc.sync.dma_start(
                            out=xw[:, dt, :],
                            in_=xT[dt * 128:(dt + 1) * 128,
                                   tw * TW:(tw + 1) * TW])
                    for which, dst in ((0, qt), (1, kt_sb)):
                        ps = psA.tile([128, TW], F32, tag="ps_qkv")
                        for dt in range(n_dt):
                            nc.tensor.matmul(
                                ps[0:E, :],
                                wk_sb[:, dt, which * E:(which + 1) * E],
                                xw[:, dt, :],
                                start=(dt == 0), stop=(dt == n_dt - 1))
                        nc.vector.tensor_copy(
                            dst[:, tw * TW:(tw + 1) * TW], ps[0:E, :])
                    # V^T then transpose into natural layout with ones column
                    ps = psA.tile([128, TW], F32, tag="ps_qkv")
                    for dt in range(n_dt):
                        nc.tensor.matmul(
                            ps[0:E, :], wk_sb[:, dt, 2 * E:3 * E],
                            xw[:, dt, :],
                            start=(dt == 0), stop=(dt == n_dt - 1))
                    vt_sb = scr.tile([E, TW], BF16, tag="vt")
                    nc.vector.tensor_copy(vt_sb, ps[0:E, :])
                    for sub in range(TW // 128):   # 4 k-tiles per window
                        k_idx = tw * (TW // 128) + sub
                        pst = psA.tile([128, 128], BF16, tag="ps_vt")
                        nc.tensor.transpose(
                            pst[:, 0:E], vt_sb[:, sub * 128:(sub + 1) * 128],
                            ident)
                        nc.vector.tensor_copy(
                            vb[:, k_idx, :].rearrange(
                                "p (h c) -> p h c", c=D_HEAD + 1)[:, :, 0:D_HEAD],
                            pst[:, 0:E].rearrange(
                                "p (h c) -> p h c", c=D_HEAD))

            # ---- phase B: causal attention, transposed flash-style ----
            with (
                nc.named_scope("attn"),
                tc.tile_pool(name="psS", bufs=4, space="PSUM") as psS,
                tc.tile_pool(name="psO", bufs=2, space="PSUM") as psO,
                tc.tile_pool(name="psBc", bufs=2, space="PSUM") as psBc,
                tc.tile_pool(name="scrBp", bufs=4) as scr,
                tc.tile_pool(name="scrB2", bufs=2) as scr2,
            ):
                for b in range(B):
                    for h in range(HPC):
                        hp = h * D_HEAD
                        for qm in range(S // QM):
                            q0 = b * S + qm * QM
                            n_k = 4 * qm + 4
                            ps_o = psO.tile([128, QM], F32, tag="ps_o")
                            pts = [None] * n_k

                            def emit_scores(k, b=b, hp=hp, qm=qm, q0=q0,
                                            pts=pts):
                                k0 = b * S + k * KT
                                ps_s = psS.tile([KT, QM], F32, tag="ps_s")
                                nc.tensor.matmul(
                                    ps_s[:, :],
                                    kt_sb[hp:hp + D_HEAD, k0:k0 + KT],
                                    qt[hp:hp + D_HEAD, q0:q0 + QM],
                                    start=True, stop=True)
                                pt = scr.tile([KT, QM], BF16, tag="pt")
                                if k >= 4 * qm:  # diagonal: causal mask
                                    nc.vector.scalar_tensor_tensor(
                                        out=pt, in0=ps_s, scalar=SCALE,
                                        in1=masks[k - 4 * qm],
                                        op0=mybir.AluOpType.mult,
                                        op1=mybir.AluOpType.add)
                                    nc.scalar.activation(
                                        out=pt, in_=pt,
                                        func=mybir.ActivationFunctionType.Exp)
                                else:
                                    nc.scalar.activation(
                                        out=pt, in_=ps_s,
                                        func=mybir.ActivationFunctionType.Exp,
                                        scale=SCALE)
                                pts[k] = pt

                            LA = 3   # score/exp lookahead so PE never waits
                            for k in range(min(LA, n_k)):
                                emit_scores(k)
                            for k in range(n_k):
                                if k + LA < n_k:
                                    emit_scores(k + LA)
                                nc.tensor.matmul(
                                    ps_o[0:D_HEAD + 1, :],
                                    vb[:, b * (S // KT) + k,
                                       h * (D_HEAD + 1):(h + 1) * (D_HEAD + 1)],
                                    pts[k],
                                    start=(k == 0), stop=(k == n_k - 1))
                                pts[k] = None
                            # normalize: row D_HEAD holds the denominator;
                            # fast-approx reciprocal, broadcast over 64
                            # partitions via K=1 matmul, then multiply
                            den_sb = scr2.tile([1, QM], F32, tag="den")
                            nc.vector.tensor_copy(
                                den_sb, ps_o[D_HEAD:D_HEAD + 1, :])
                            recip_sb = scr2.tile([1, QM], F32, tag="recip")
                            nc.vector.reciprocal_approx_fast(
                                out=recip_sb, in_=den_sb)
                            ps_b = psBc.tile([D_HEAD, QM], F32, tag="ps_b")
                            nc.tensor.matmul(
                                ps_b[:, :], ones_row, recip_sb,
                                start=True, stop=True)
                            num_sb = scr2.tile([D_HEAD, QM], F32R, tag="num")
                            nc.vector.tensor_copy(num_sb, ps_o[0:D_HEAD, :])
                            attn_sb = scr2.tile([D_HEAD, QM], F32R, tag="attn")
                            nc.vector.tensor_mul(attn_sb, num_sb, ps_b)
                            shard = b * 2 + qm // 2
                            qc = (qm % 2) * QM
                            nc.sync.dma_start(
                                out=cc_in[shard, hp:hp + D_HEAD, qc:qc + QM],
                                in_=attn_sb)

            # ---- phase C: A2A reshard + output projection ----
            with nc.named_scope("a2a"):
                nc.gpsimd.collective_compute(
                    "AllToAll", mybir.AluOpType.bypass,
                    ins=[cc_in.opt()], outs=[cc_out.opt()], replica_groups=rg)

            with (
                nc.named_scope("oproj"),
                tc.tile_pool(name="psC", bufs=3, space="PSUM") as psC,
                tc.tile_pool(name="spC", bufs=2) as sp,
                tc.tile_pool(name="scrC", bufs=2) as scr,
            ):
                for tt in range(TOK // 128):
                    a_sb = sp.tile([128, n_dt, 128], F32R, tag="a")
                    for dt in range(n_dt):
                        nc.sync.dma_start(
                            out=a_sb[:, dt, :],
                            in_=cc_out[dt, :, tt * 128:(tt + 1) * 128])
                    for et in range(D // 512):
                        ps = psC.tile([128, 512], F32, tag="ps_c")
                        for dt in range(n_dt):
                            nc.tensor.matmul(
                                ps[:, :], a_sb[:, dt, :],
                                wo_sb[:, dt, et * 512:(et + 1) * 512],
                                start=(dt == 0), stop=(dt == n_dt - 1))
                        o_sb = scr.tile([128, 512], F32, tag="o")
                        nc.vector.tensor_copy(o_sb, ps)
                        nc.sync.dma_start(
                            out=out[tt * 128:(tt + 1) * 128,
                                    et * 512:(et + 1) * 512],
                            in_=o_sb)
    nc.compile()
    return nc


def kernel(x, W_qkv, W_o):
    global _CACHED_NC, LAST_EXEC_NS
    if _CACHED_NC is None:
        _CACHED_NC = _build()
    nc = _CACHED_NC

    x = np.ascontiguousarray(x, dtype=np.float32)
    xT = np.ascontiguousarray(x.reshape(T, D).T)          # (1024, 8192)
    woT = np.ascontiguousarray(W_o.astype(np.float32).T)  # (1024, 1024)
    in_maps = []
    for i in range(N_CORES):
        rows = []
        for blk in range(3):                               # Q, K, V blocks
            rows.append(W_qkv[blk * D + i * E: blk * D + (i + 1) * E, :])
        wqkvT = np.ascontiguousarray(
            np.concatenate(rows, axis=0).astype(np.float32).T)  # (1024, 384)
        in_maps.append({"xT": xT, "wqkvT": wqkvT, "woT": woT})

    res = run_bass_kernel_spmd(nc, in_maps, core_ids=list(range(N_CORES)),
                               trace=TRACE)
    LAST_EXEC_NS = res.exec_time_ns
    kernel.LAST_RES = res
    full = np.empty((T, D), dtype=np.float32)
    for i in range(N_CORES):
        full[i * TOK:(i + 1) * TOK, :] = res.results[i]["out"]
    return full.reshape(B, S, D)
